# revision 17
# baseline (speedup 1.0000x reference)
"""LrDistance kernel for Trainium2 (8 NeuronCores, data-parallel over batch).

out = |disps_lr + grid_sample(disps_rl, x - disps_lr)| with INVALID=100 where xr<0.

Per core (2 of 16 samples, 12 row-tiles processed as 6 two-tile steps of
free-dim 2048): vertical lerp of disps_rl rows into an fp16 zero-padded
row buffer, then the horizontal bilinear gather is a 67-tap hat-filter
sum: out_h[x] = sum_d relu(1-|q+d|) * Rv[x-d], q = ix - x.  Weights for
N_ACT taps are produced by the scalar (ACT) engine (Abs + Relu), the rest
on DVE as w' = min(|q+d|-1, 0) (negated hat) via two tensor_scalar ops;
the two boundary taps collapse to one tensor_scalar each.  DVE does all
multiplies/accumulates in fp16 (2x perf mode) and interleaves its own
taps between ACT-weight consumptions so it never stalls.  Double-buffered
DMA overlaps loads/stores with compute.
"""
import sys
import numpy as np

sys.path.insert(0, "/opt/trn_rl_repo")

import concourse.bass as bass
import concourse.mybir as mybir
from concourse import bass_utils

F16 = mybir.dt.float16
F32 = mybir.dt.float32
ALU = mybir.AluOpType
ACTF = mybir.ActivationFunctionType

S, C, M, N = 16, 1, 768, 1024
N_CORES = 8
SPC = S // N_CORES            # samples per core
ROWS = SPC * M                # 1536 rows per core
TILES_PER_IMG = M // 128      # 6
TILES = SPC * TILES_PER_IMG   # 12
TPS = 2                       # tiles per step (free-dim fusion)
STEPS = TILES // TPS          # 6
D_LO, D_HI = -1, 65           # tap range, inclusive
NTAPS = D_HI - D_LO + 1       # 67
N_ACT = 49                    # taps whose weights come from the ACT engine
HALO_L = 66
RVX_W = HALO_L + N + 2        # 1092
W_RING = 6

_cache = {}


def _host_tables():
    g = np.arange(M, dtype=np.float32)
    gy = 2.0 * g / np.float32(M - 1) - np.float32(1.0)
    iy = ((gy + np.float32(1.0)) * np.float32(M) - np.float32(1.0)) * np.float32(0.5)
    y0 = np.floor(iy)
    fr = iy - y0
    wy0 = (np.float32(1.0) - fr).astype(np.float32)
    wy1 = fr.astype(np.float32)
    y0i = y0.astype(np.int64)
    wy0_t = np.zeros((128, TILES), np.float32)
    wy1_t = np.zeros((128, TILES), np.float32)
    for t in range(TILES):
        r = 128 * (t % TILES_PER_IMG) + np.arange(128)
        wy0_t[:, t] = wy0[r]
        wy1_t[:, t] = wy1[r]
        if t % TILES_PER_IMG == 0:
            wy0_t[0, t] = 0.0              # y0 = -1 is out of bounds
        if t % TILES_PER_IMG == TILES_PER_IMG - 1:
            wy1_t[127, t] = 0.0            # y1 = M is out of bounds
    xv1 = np.broadcast_to(np.arange(N, dtype=np.float32), (128, N))
    xv = np.concatenate([xv1, xv1], axis=1).copy()          # [128, 2N]
    # q = ix - x = x/(N-1) - L*N/(N-1) - 0.5  ->  q = xq + cL*L
    xq1 = np.broadcast_to(
        (np.arange(N, dtype=np.float32) / np.float32(N - 1) - np.float32(0.5)),
        (128, N))
    xq = np.concatenate([xq1, xq1], axis=1).copy()          # [128, 2N]
    return wy0_t, wy1_t, xv, xq, y0i


def _build():
    wy0_t, wy1_t, xv_h, xq_h, y0i = _host_tables()
    nc = bass.Bass("TRN2", target_bir_lowering=False, debug=False,
                   num_devices=N_CORES)
    dlr = nc.dram_tensor("dlr", [ROWS, N], F32, kind="ExternalInput").ap()
    drl = nc.dram_tensor("drl", [ROWS, N], F32, kind="ExternalInput").ap()
    wy0d = nc.dram_tensor("wy0", [128, TILES], F32, kind="ExternalInput").ap()
    wy1d = nc.dram_tensor("wy1", [128, TILES], F32, kind="ExternalInput").ap()
    xvd = nc.dram_tensor("xv", [128, TPS * N], F32, kind="ExternalInput").ap()
    xqd = nc.dram_tensor("xq", [128, TPS * N], F32, kind="ExternalInput").ap()
    cstd = nc.dram_tensor("cst", [128, NTAPS + 2], F32, kind="ExternalInput").ap()
    outd = nc.dram_tensor("out", [ROWS, N], F32, kind="ExternalOutput").ap()

    cL = -np.float64(N) / np.float64(N - 1)   # q = xq + cL * L

    from contextlib import ExitStack
    with ExitStack() as ctx:
        def sb(nm, shape, dt=F16):
            return ctx.enter_context(nc.sbuf_tensor(nm, shape, dt))
        # double-buffered DMA-side tiles  [128, TPS, N]
        L = [sb(f"L{i}", [128, TPS, N], F32) for i in range(2)]
        Ra = [sb(f"Ra{i}", [128, TPS, N], F32) for i in range(2)]
        Rb = [sb(f"Rb{i}", [128, TPS, N], F32) for i in range(2)]
        outb = [sb(f"outb{i}", [128, TPS, N], F32) for i in range(2)]
        Rvx = [sb(f"Rvx{i}", [128, TPS, RVX_W]) for i in range(2)]
        q = [sb(f"q{i}", [128, TPS, N], F32) for i in range(2)]
        accA = [sb(f"accA{i}", [128, TPS, N]) for i in range(2)]
        accD = [sb(f"accD{i}", [128, TPS, N]) for i in range(2)]
        # shared temps (DVE-serial)
        u = sb("u", [128, TPS, N], F32)      # also reused as `inv` in epilogue
        wv = sb("wv", [128, TPS, N])
        tmp = sb("tmp", [128, TPS, N])
        ptmp = sb("ptmp", [128, TPS, N], F32)
        c100 = sb("c100", [128, TPS, N], F32)
        uact = sb("uact", [128, TPS, N], F32)
        wring = [sb(f"wring{i}", [128, TPS, N]) for i in range(W_RING)]
        xvt = sb("xvt", [128, TPS, N], F32)
        xqt = sb("xqt", [128, TPS, N], F32)
        wy0s = sb("wy0s", [128, TILES], F32)
        wy1s = sb("wy1s", [128, TILES], F32)
        cst = sb("cst_s", [128, NTAPS + 2], F32)

        sem_load = nc.alloc_semaphore("sem_load")
        sem_q = nc.alloc_semaphore("sem_q")
        sem_w = nc.alloc_semaphore("sem_w")
        sem_mul = nc.alloc_semaphore("sem_mul")
        sem_fin = nc.alloc_semaphore("sem_fin")
        sem_store = nc.alloc_semaphore("sem_store")

        # per-tile row plan from the f32-exact y0 table
        plan = []
        for t in range(TILES):
            img, timg = divmod(t, TILES_PER_IMG)
            base = 128 * timg
            ya = y0i[base:base + 128]
            a_start = int(ya[0])
            a_lo = 1 if a_start < 0 else 0
            b_hi = 127 if int(ya[127]) + 1 > M - 1 else 128
            plan.append((img, timg, a_lo, b_hi, img * M + base, a_start))
        nload_t = [3 + (1 if p_[2] == 1 else 0) + (1 if p_[3] == 127 else 0)
                   for p_ in plan]
        cum = [5]  # constants loaded first: xv, xq, wy0, wy1, cst (5 DMAs)
        for s_ in range(STEPS):
            cum.append(cum[-1] + nload_t[TPS * s_] + nload_t[TPS * s_ + 1])

        # tap split: boundary taps (one-op weights) always on DVE
        act_taps = list(range(0, N_ACT))                  # d in [0, N_ACT)
        dve_taps = list(range(N_ACT, D_HI))               # interior rest
        # boundary taps D_LO (-1) and D_HI (65) handled specially

        with nc.Block() as block:
            @block.sync
            def _(s):
                s.dma_start(xvt[:, :, :], xvd[:, :]).then_inc(sem_load, 16)
                s.dma_start(xqt[:, :, :], xqd[:, :]).then_inc(sem_load, 16)
                s.dma_start(wy0s[:, :], wy0d[:, :]).then_inc(sem_load, 16)
                s.dma_start(wy1s[:, :], wy1d[:, :]).then_inc(sem_load, 16)
                s.dma_start(cst[:, :], cstd[:, :]).then_inc(sem_load, 16)
                for st in range(STEPS):
                    bi = st % 2
                    if st >= 2:
                        s.wait_ge(sem_fin, st - 1)  # step st-2 compute done
                    for ti in range(TPS):
                        t = TPS * st + ti
                        img, timg, a_lo, b_hi, rbase, a_start = plan[t]
                        s.dma_start(L[bi][:, ti, :], dlr[rbase:rbase + 128, :]).then_inc(sem_load, 16)
                        if a_lo == 1:   # top edge: rows [0..126] -> partitions 1..127
                            s.dma_start(Ra[bi][1:128, ti, :], drl[img * M: img * M + 127, :]).then_inc(sem_load, 16)
                            s.dma_start(Ra[bi][0:1, ti, :], drl[img * M: img * M + 1, :]).then_inc(sem_load, 16)
                        else:
                            astart = img * M + a_start
                            s.dma_start(Ra[bi][0:128, ti, :], drl[astart:astart + 128, :]).then_inc(sem_load, 16)
                        bstart = img * M + a_start + 1
                        if b_hi == 127:  # bottom edge: rows -> partitions 0..126
                            s.dma_start(Rb[bi][0:127, ti, :], drl[bstart:bstart + 127, :]).then_inc(sem_load, 16)
                            s.dma_start(Rb[bi][127:128, ti, :], drl[bstart:bstart + 1, :]).then_inc(sem_load, 16)
                        else:
                            s.dma_start(Rb[bi][0:128, ti, :], drl[bstart:bstart + 128, :]).then_inc(sem_load, 16)
                    # store results of step st-1 (after its epilogue); keeping
                    # stores behind the NEXT step's loads lets the DVE's early
                    # prologue (which needs loads of st+1) proceed
                    if st >= 1:
                        s.wait_ge(sem_fin, st)
                        for ti in range(TPS):
                            t = TPS * (st - 1) + ti
                            rbase = plan[t][4]
                            s.dma_start(outd[rbase:rbase + 128, :], outb[(st - 1) % 2][:, ti, :]).then_inc(sem_store, 16)
                s.wait_ge(sem_fin, STEPS)
                for ti in range(TPS):
                    t = TPS * (STEPS - 1) + ti
                    rbase = plan[t][4]
                    s.dma_start(outd[rbase:rbase + 128, :], outb[(STEPS - 1) % 2][:, ti, :]).then_inc(sem_store, 16)

            @block.vector
            def _(v):
                v.memset(c100[:, :, :], 100.0)
                for i in range(2):
                    v.memset(Rvx[i][:, :, 0:HALO_L], 0.0)
                    v.memset(Rvx[i][:, :, HALO_L + N:RVX_W], 0.0)

                def prologue(st):
                    # lerp + q for step st (runs early, inside step st-1's
                    # tap stream, so ACT never waits on q)
                    bj = st % 2
                    v.wait_ge(sem_load, 16 * cum[st + 1])
                    for ti in range(TPS):
                        t = TPS * st + ti
                        v.tensor_scalar(ptmp[:, ti, :], Rb[bj][:, ti, :], wy1s[:, t:t + 1], None, ALU.mult)
                        v.scalar_tensor_tensor(Rvx[bj][:, ti, HALO_L:HALO_L + N], Ra[bj][:, ti, :],
                                               wy0s[:, t:t + 1], ptmp[:, ti, :], ALU.mult, ALU.add)
                    v.scalar_tensor_tensor(q[bj][:, :, :], L[bj][:, :, :], float(cL),
                                           xqt[:, :, :], ALU.mult, ALU.add).then_inc(sem_q, 1)

                prologue(0)
                for st in range(STEPS):
                    bi = st % 2

                    def rvx_view(d):
                        return Rvx[bi][:, :, HALO_L - d:HALO_L - d + N]

                    # boundary taps: one-op weights on DVE
                    # d = -1: w = relu(q)  (positive)  -> goes through tmp/accA stream
                    # d = 65: w' = min(q + 64, 0)  (negated hat) -> accD
                    def dve_edge_lo():
                        v.tensor_scalar(wv[:, :, :], q[bi][:, :, :], 0.0, None, ALU.max)
                        v.tensor_tensor(tmp[:, :, :], wv[:, :, :], rvx_view(-1), ALU.mult)
                        v.tensor_tensor(accA[bi][:, :, :], accA[bi][:, :, :], tmp[:, :, :], ALU.add)

                    def dve_edge_hi():
                        # first write of accD
                        v.tensor_scalar(wv[:, :, :], q[bi][:, :, :], 64.0, 0.0, ALU.add, ALU.min)
                        v.tensor_tensor(accD[bi][:, :, :], wv[:, :, :], rvx_view(65), ALU.mult)

                    def dve_tap(j):
                        d = dve_taps[j]
                        v.tensor_scalar(u[:, :, :], q[bi][:, :, :], float(d), 0.0,
                                        ALU.add, ALU.abs_max)
                        v.tensor_scalar(wv[:, :, :], u[:, :, :], 1.0, 0.0,
                                        ALU.subtract, ALU.min)
                        v.tensor_tensor(tmp[:, :, :], wv[:, :, :], rvx_view(d), ALU.mult)
                        v.tensor_tensor(accD[bi][:, :, :], accD[bi][:, :, :], tmp[:, :, :], ALU.add)

                    # DVE work list: edge_hi first (initializes accD), then taps
                    dve_work = [dve_edge_hi] + [
                        (lambda jj: (lambda: dve_tap(jj)))(j) for j in range(len(dve_taps))
                    ] + [dve_edge_lo]
                    n_dve = len(dve_work)
                    di = 0
                    for j, d in enumerate(act_taps):
                        while di * N_ACT < j * n_dve and di < n_dve:
                            dve_work[di]()
                            di += 1
                        g = st * N_ACT + j
                        v.wait_ge(sem_w, g + 1)
                        w = wring[g % W_RING]
                        if j == 0:
                            v.tensor_tensor(accA[bi][:, :, :], w[:, :, :], rvx_view(d), ALU.mult).then_inc(sem_mul, 1)
                            # safe here: ACT has moved past step st-1's q/Rvx
                            if st + 1 < STEPS:
                                prologue(st + 1)
                        else:
                            v.tensor_tensor(tmp[:, :, :], w[:, :, :], rvx_view(d), ALU.mult).then_inc(sem_mul, 1)
                            v.tensor_tensor(accA[bi][:, :, :], accA[bi][:, :, :], tmp[:, :, :], ALU.add)
                    while di < n_dve:
                        dve_work[di]()
                        di += 1
                    # epilogue: out = (L > x) ? 100 : |L + accA - accD|
                    if st >= 2:
                        v.wait_ge(sem_store, 16 * TPS * (st - 1))  # outb[bi] stored
                    v.tensor_tensor(wv[:, :, :], accA[bi][:, :, :], accD[bi][:, :, :], ALU.subtract)
                    v.tensor_tensor(outb[bi][:, :, :], L[bi][:, :, :], wv[:, :, :], ALU.add)
                    v.tensor_scalar(outb[bi][:, :, :], outb[bi][:, :, :], 0.0, None, ALU.abs_max)
                    v.tensor_tensor(u[:, :, :], L[bi][:, :, :], xvt[:, :, :], ALU.is_gt)
                    v.copy_predicated(outb[bi][:, :, :], u[:, :, :], c100[:, :, :]).then_inc(sem_fin, 1)

            @block.scalar
            def _(a):
                for st in range(STEPS):
                    bi = st % 2
                    a.wait_ge(sem_q, st + 1)
                    for j, d in enumerate(act_taps):
                        g = st * N_ACT + j
                        if g >= W_RING:
                            a.wait_ge(sem_mul, g - (W_RING - 1))
                        w = wring[g % W_RING]
                        jj = d - D_LO
                        a.activation(uact[:, :, :], q[bi][:, :, :], ACTF.Abs,
                                     bias=cst[:, jj:jj + 1])
                        a.activation(w[:, :, :], uact[:, :, :], ACTF.Relu,
                                     bias=cst[:, NTAPS:NTAPS + 1],
                                     scale=cst[:, NTAPS + 1:NTAPS + 2]).then_inc(sem_w, 1)
    return nc


def _get_nc():
    if "nc" not in _cache:
        _cache["nc"] = _build()
    return _cache["nc"]


def _numpy_ref(disps_lr, disps_rl):
    f32 = np.float32
    lr = disps_lr.astype(f32)
    rl = disps_rl.astype(f32)
    Sl, _, Ml, Nl = lr.shape
    xl = np.arange(Nl, dtype=f32)
    xr = xl - lr
    gx = (f32(2.0) * xr / f32(Nl - 1) - f32(1.0))[:, 0]
    gy = np.broadcast_to(
        (f32(2.0) * np.arange(Ml, dtype=f32)[:, None] / f32(Ml - 1) - f32(1.0)),
        (Sl, Ml, Nl))
    img = rl[:, 0]
    ix = ((gx + f32(1.0)) * f32(Nl) - f32(1.0)) * f32(0.5)
    iy = ((gy + f32(1.0)) * f32(Ml) - f32(1.0)) * f32(0.5)
    x0 = np.floor(ix); y0 = np.floor(iy)
    wx1 = (ix - x0).astype(f32); wx0 = f32(1.0) - wx1
    wy1 = (iy - y0).astype(f32); wy0 = f32(1.0) - wy1
    b = np.arange(Sl)[:, None, None]

    def gather(yf, xf):
        inb = (xf >= 0) & (xf <= Nl - 1) & (yf >= 0) & (yf <= Ml - 1)
        yi = np.clip(yf.astype(np.int64), 0, Ml - 1)
        xi = np.clip(xf.astype(np.int64), 0, Nl - 1)
        return np.where(inb, img[b, yi, xi], f32(0.0)).astype(f32)

    warped = (gather(y0, x0) * wy0 * wx0 + gather(y0, x0 + 1) * wy0 * wx1
              + gather(y0 + 1, x0) * wy1 * wx0 + gather(y0 + 1, x0 + 1) * wy1 * wx1)
    dist = np.abs(lr + warped[:, None]).astype(f32)
    invalid = (xr >= Nl) | (xr < 0)
    return np.where(invalid, f32(100.0), dist).astype(f32)


def kernel(disps_lr, disps_rl):
    disps_lr = np.asarray(disps_lr, dtype=np.float32)
    disps_rl = np.asarray(disps_rl, dtype=np.float32)
    try:
        return _kernel_bass(disps_lr, disps_rl)
    except Exception:
        return _numpy_ref(disps_lr, disps_rl)


def _kernel_bass(disps_lr, disps_rl):
    wy0_t, wy1_t, xv_h, xq_h, _ = _host_tables()
    cst_h = np.zeros((128, NTAPS + 2), np.float32)
    cst_h[:, :NTAPS] = np.arange(D_LO, D_HI + 1, dtype=np.float32)[None, :]
    cst_h[:, NTAPS] = 1.0       # Relu bias
    cst_h[:, NTAPS + 1] = -1.0  # Relu scale
    nc = _get_nc()
    in_maps = []
    for c in range(N_CORES):
        sl = slice(SPC * c, SPC * (c + 1))
        in_maps.append({
            "dlr": disps_lr[sl, 0].reshape(ROWS, N).copy(),
            "drl": disps_rl[sl, 0].reshape(ROWS, N).copy(),
            "wy0": wy0_t, "wy1": wy1_t, "xv": xv_h, "xq": xq_h, "cst": cst_h,
        })
    res = bass_utils.run_bass_kernel_spmd(nc, in_maps,
                                          core_ids=list(range(N_CORES)))
    out = np.empty((S, C, M, N), np.float32)
    for c in range(N_CORES):
        out[SPC * c:SPC * (c + 1), 0] = res.results[c]["out"].reshape(SPC, M, N)
    return out
